# revision 3
# baseline (speedup 1.0000x reference)
"""Trainium2 Bass kernel for the ExportableStudentSNN1d problem (fp8 conv2).

Data-parallel over batch: 64 samples -> 8 cores x 8 samples. Each core runs
an identical NEFF on its batch shard; host concatenates the [8, 4] outputs.

Math notes (TAU1 = 1.0 makes layer-1 LIF memoryless):
  s1_t = (conv1(x_t)*G + b1*G >= TH1)        <=> conv1(x_t) >= TH1/G - b1
  s1 is stored +/-1-encoded: sgn = Sign(conv1 - th1) (ACT engine), and
  since s1 = (sgn+1)/2 is linear, conv2 runs on sgn with HALVED weights
  (exact power-of-2 scale in fp8) and 0.5*sum(W2q) folded into b2p.
  layer2 runs in a 2^11-scaled domain (all linear + thresholds scale):
     psum2 = conv2 with W2 pre-scaled by (10/9)*G*2^11/2, fp8-e4m3
     y     = (psum2 + b2p) + carry           (b2p, carry, TH2 all 2^11-scaled)
     m'    = (y < TH2') * (-1/9);  carry' = y * m'
     spike counts via ACT Sign(y - TH2') accumulation.
  fp8 weight-quantization DC error is cancelled by folding
  -sum_{ci,k} dW[co,ci,k] * E[s1[ci]] into b2 (E[s1[ci]] analytic: conv1
  output is Gaussian with sigma = ||W1[ci]||_F).
  out[b,c] = (sum_{t,l} sp)/(T*L) @ Wfc.T + bfc

conv2 uses fp8 DoubleRow matmuls: taps (0,1),(2,3),(4,5),(6,7) run as 4
pair-matmuls (2 MACs/cell/cycle), tap 8 as a normal fp8 matmul. The s1
tile holds two slabs [128, 2, S1P]: slab1 = slab0 shifted one column (DMA
SBUF->SBUF copy), so a pair's two taps read the same offset in both slabs.
conv1 stays bf16 with a DMA-materialized im2col.

Engine budget per (seg,t) (keeps every engine under the ~5.2us Tensor
time): DVE does y-stt + carry-mult per 512-chunk, GpSimd the m mask op per
chunk, ACT the two spike-count Signs + the s1 slab0 Sign, DMA the slab1
shift copy.
"""

import math

import numpy as np
import ml_dtypes

import concourse.bacc as bacc
import concourse.tile as tile
import concourse.mybir as mybir
from concourse.bass_utils import run_bass_kernel_spmd

F32 = mybir.dt.float32
BF16 = mybir.dt.bfloat16
FP8 = mybir.dt.float8e4
E4 = ml_dtypes.float8_e4m3

N_CORES = 8
B, C_IN, L, T = 64, 12, 2048, 20
C1, C2, K, PAD = 128, 256, 9, 4
GAIN, TAU2, TH1, TH2 = 3.0, 0.9, 0.02, 0.02
NCLS = 4
B_SH = B // N_CORES            # 8 samples per core
LH = 1024                      # L processed in halves
HALO = 8                       # x halo per side (conv1 then conv2 shifts)
S1W = LH + 2 * PAD             # 1032 s1 columns needed per L-half
S1P = 1040                     # s1 slab width (mult of 16 for DR pair step)
XW = LH + 2 * HALO             # 1040 x columns staged per L-half
A2S = (10.0 / 9.0) * GAIN      # 10/3: multiplier on conv2 psum
SC2 = 2.0 ** 11                # fp8 range scale; layer-2 runs 2^11-scaled
TH2S = TH2 * SC2
MDECAY = 1.0 / 9.0

_CACHE = {}


def _build():
    nc = bacc.Bacc("TRN2", target_bir_lowering=False, debug=False)

    x_d = nc.dram_tensor("x", [B_SH, C_IN, T, L], BF16, kind="ExternalInput")
    w1t_d = nc.dram_tensor("w1t", [K * C_IN, C1], BF16, kind="ExternalInput")
    w2dr_d = nc.dram_tensor("w2dr", [C1, 2, 8 * C1], FP8, kind="ExternalInput")
    w28_d = nc.dram_tensor("w28", [C1, C2], FP8, kind="ExternalInput")
    nth1_d = nc.dram_tensor("nth1", [C1, 1], F32, kind="ExternalInput")
    b2p_d = nc.dram_tensor("b2p", [C1, 2], F32, kind="ExternalInput")
    wfc_d = nc.dram_tensor("wfc", [C1, 2 * NCLS], F32, kind="ExternalInput")
    bfc_d = nc.dram_tensor("bfc", [NCLS, 1], F32, kind="ExternalInput")
    out_d = nc.dram_tensor("out", [B_SH, NCLS], F32, kind="ExternalOutput")

    with tile.TileContext(nc) as tc:
        with (
            tc.tile_pool(name="const", bufs=1) as cpool,
            tc.tile_pool(name="xstage", bufs=2) as xpool,
            tc.tile_pool(name="s1", bufs=2) as s1pool,
            tc.tile_pool(name="lif", bufs=3) as lifpool,
            tc.tile_pool(name="carry", bufs=2) as cpool2,
            tc.tile_pool(name="psum1", bufs=1, space="PSUM") as pp1,
            tc.tile_pool(name="psum2", bufs=2, space="PSUM") as pp2,
            tc.tile_pool(name="psfc", bufs=1, space="PSUM") as ppfc,
        ):
            # ---- constants / weights (resident) ----
            # w1t rows (12k+ci) hold W1[:, ci, k] (im2col layout)
            w1t = cpool.tile([K * C_IN, C1], BF16)
            nc.sync.dma_start(w1t[:], w1t_d.ap())
            # DR pairs: w2dr[ci, i, (j*2+h)*128+co] = W2s[h*128+co, ci, 2j+i]
            w2dr = cpool.tile([C1, 2, 8 * C1], FP8)
            nc.sync.dma_start(w2dr[:], w2dr_d.ap())
            # tap 8: w28[ci, h*128+co] = W2s[h*128+co, ci, 8]
            w28 = cpool.tile([C1, C2], FP8)
            nc.sync.dma_start(w28[:], w28_d.ap())
            nth1 = cpool.tile([C1, 1], F32)
            nc.sync.dma_start(nth1[:], nth1_d.ap())
            b2p = cpool.tile([C1, 2], F32)
            nc.sync.dma_start(b2p[:], b2p_d.ap())
            nth2 = cpool.tile([C1, 1], F32)
            nc.gpsimd.memset(nth2[:], -TH2S)
            wfc = cpool.tile([C1, 2 * NCLS], F32)
            nc.sync.dma_start(wfc[:], wfc_d.ap())
            bfc = cpool.tile([NCLS, 1], F32)
            nc.sync.dma_start(bfc[:], bfc_d.ap())
            # spike counts, one column per (h, b, lh, t)
            acc = cpool.tile([C1, 2 * B_SH * 2 * T], F32)

            segs = [(b, lh) for b in range(B_SH) for lh in range(2)]

            def stage_segment(idx):
                # im2col staging: rows (12k+ci) = x[ci] shifted by tap k.
                # column (t, c) of row-group k = x[b, ci, t, l0+c+k-8]
                b, lh = segs[idx]
                l0 = lh * LH
                xs = xpool.tile([K * C_IN, T * S1W], BF16)
                xsv = xs[:].rearrange("p (t c) -> p t c", c=S1W)
                # zero the possible halo bands (32-aligned base partition
                # required for engine ops -> memset all rows; the DMAs
                # below overwrite whatever is valid)
                if l0 == 0:
                    nc.gpsimd.memset(xsv[:, :, 0:HALO], 0.0)
                if l0 + LH == L:
                    nc.gpsimd.memset(xsv[:, :, S1W - HALO : S1W], 0.0)
                for k in range(K):
                    rows = slice(C_IN * k, C_IN * (k + 1))
                    c_lo = max(0, HALO - k - l0)
                    c_hi = min(S1W, L - l0 - k + HALO)
                    src = x_d.ap()[b, :, :,
                                   l0 + c_lo + k - HALO : l0 + c_hi + k - HALO]
                    if idx == 0:
                        # cold start: split so the first timesteps' columns
                        # land first
                        nc.sync.dma_start(
                            xsv[rows, 0:2, c_lo:c_hi], src[:, 0:2, :])
                        nc.sync.dma_start(
                            xsv[rows, 2:T, c_lo:c_hi], src[:, 2:T, :])
                    else:
                        nc.sync.dma_start(xsv[rows, :, c_lo:c_hi], src)
                carry = cpool2.tile([C1, 2 * LH], F32)
                nc.gpsimd.memset(carry[:], 0.0)
                return xs, carry

            def conv1_block(xs, t):
                # conv1: one K=108 matmul per chunk. s1 slab0 = Sign(p1-th1)
                # on ACT (+/-1 encoding); slab1 = slab0 shifted one column
                # via SBUF->SBUF DMA. Cols >= 1032 are junk (stale psum ->
                # +/-1) but are never read by the conv2 matmuls.
                p1 = pp1.tile([C1, 1536], F32)
                for c0, cn in ((0, 512), (512, 512), (1024, S1W - 1024)):
                    nc.tensor.matmul(
                        p1[:, c0 : c0 + cn],
                        w1t[:],
                        xs[:, t * S1W + c0 : t * S1W + c0 + cn],
                        start=True,
                        stop=True,
                    )
                s1 = s1pool.tile([C1, 2, S1P], FP8)
                nc.scalar.activation(
                    s1[:, 0], p1[:, 0:S1P],
                    mybir.ActivationFunctionType.Sign,
                    bias=nth1[:],
                )
                nc.sync.dma_start(s1[:, 1, 0 : S1P - 1], s1[:, 0, 1:S1P])
                return s1

            def conv2_block(s1, h):
                p2 = pp2.tile([C1, LH], F32)
                for c0 in (0, 512):
                    for j in range(4):
                        nc.tensor.matmul(
                            p2[:, c0 : c0 + 512],
                            w2dr[:, 0:2, (j * 2 + h) * C1 : (j * 2 + h + 1) * C1],
                            s1[:, 0:2, c0 + 2 * j : c0 + 2 * j + 512],
                            start=(j == 0),
                            stop=False,
                            perf_mode=mybir.MatmulPerfMode.DoubleRow,
                        )
                    nc.tensor.matmul(
                        p2[:, c0 : c0 + 512],
                        w28[:, h * C1 : (h + 1) * C1],
                        s1[:, 0, c0 + 8 : c0 + 8 + 512],
                        start=False,
                        stop=True,
                    )
                return p2

            def lif_front(carry, p2, h):
                # per 512-chunk: y = (psum2 + b2p) + carry (DVE),
                # m = (y < TH2')*(-1/9) (GpSimd). Chunked so the
                # y->m->carry chain pipelines under the PE.
                ch = carry[:, h * LH : (h + 1) * LH]
                y = lifpool.tile([C1, LH], F32, tag="y")
                m = lifpool.tile([C1, LH], F32, tag="m")
                for q in (0, 512):
                    nc.vector.scalar_tensor_tensor(
                        y[:, q : q + 512], p2[:, q : q + 512],
                        b2p[:, h : h + 1], ch[:, q : q + 512],
                        op0=mybir.AluOpType.add, op1=mybir.AluOpType.add,
                    )
                    nc.gpsimd.tensor_scalar(
                        m[:, q : q + 512], y[:, q : q + 512], TH2S, -MDECAY,
                        op0=mybir.AluOpType.is_lt, op1=mybir.AluOpType.mult,
                    )
                return y, m

            def lif_back(carry, y, m, h, col):
                # sign-sum for spike counting (off the carry chain)
                sg = lifpool.tile([C1, LH], F32, tag="sg")
                nc.scalar.activation(
                    sg[:], y[:], mybir.ActivationFunctionType.Sign,
                    bias=nth2[:],
                    accum_out=acc[:, col + h * (B_SH * 2 * T) :
                                  col + h * (B_SH * 2 * T) + 1],
                )
                # carry chain tail on DVE (cheap SBUF-only multiply)
                ch = carry[:, h * LH : (h + 1) * LH]
                for q in (0, 512):
                    nc.vector.tensor_tensor(
                        ch[:, q : q + 512], y[:, q : q + 512],
                        m[:, q : q + 512], op=mybir.AluOpType.mult,
                    )

            # conv1 of segment idx+1's t=0 fills the empty t=19 pipeline
            # slot of segment idx, so segment boundaries don't stall PE
            staged = stage_segment(0)
            s1_cur = conv1_block(staged[0], 0)
            for idx in range(len(segs)):
                b, lh = segs[idx]
                xs, carry = staged
                if idx + 1 < len(segs):
                    staged = stage_segment(idx + 1)
                for t in range(T):
                    col = b * (2 * T) + lh * T + t
                    p2_0 = conv2_block(s1_cur, 0)
                    # emit conv1(t+1)+s1(t+1) before the h0 LIF ops: PE order
                    # is unchanged, but s1 lands ~2us earlier so conv2(t+1,h0)
                    # never waits on it
                    if t + 1 < T:
                        s1_next = conv1_block(xs, t + 1)
                    elif idx + 1 < len(segs):
                        s1_next = conv1_block(staged[0], 0)
                    else:
                        s1_next = None
                    y0, m0 = lif_front(carry, p2_0, 0)
                    lif_back(carry, y0, m0, 0, col)
                    p2_1 = conv2_block(s1_cur, 1)
                    y1, m1 = lif_front(carry, p2_1, 1)
                    lif_back(carry, y1, m1, 1, col)
                    s1_cur = s1_next

            # ---- pooling + FC head ----
            pooled = cpool.tile([C1, 2 * B_SH], F32)
            nc.vector.tensor_reduce(
                pooled[:],
                acc[:].rearrange("p (h b c) -> p (h b) c", h=2, b=B_SH),
                axis=mybir.AxisListType.X, op=mybir.AluOpType.add,
            )
            pfc = ppfc.tile([NCLS, B_SH], F32)
            for h in range(2):
                nc.tensor.matmul(
                    pfc[:],
                    wfc[:, h * NCLS : (h + 1) * NCLS],
                    pooled[:, h * B_SH : (h + 1) * B_SH],
                    start=(h == 0),
                    stop=(h == 1),
                )
            # pfc holds Wfc @ sign_sums; counts = (sign_sum + T*L)/2 is folded
            # into scale and the host-adjusted bias
            fin = cpool.tile([NCLS, B_SH], F32)
            nc.scalar.activation(
                fin[:], pfc[:], mybir.ActivationFunctionType.Identity,
                bias=bfc[:], scale=1.0 / float(2 * T * L),
            )
            nc.sync.dma_start(out_d.ap().rearrange("b c -> c b"), fin[:])

    nc.compile()
    return nc


def _prep_consts(W1, b1, W2, b2, Wfc, bfc):
    # w1t im2col layout: row (12k+ci), col co = W1[co, ci, k]
    w1t = np.ascontiguousarray(W1.transpose(2, 1, 0)).reshape(K * C_IN, C1)
    # W2 pre-scaled to the 2^11 domain, fp8-e4m3 quantized, then HALVED
    # (exact in fp8) for the +/-1 s1 encoding
    w2q8 = (W2.astype(np.float64) * (A2S * SC2)).astype(np.float32).astype(E4)
    w2h = (w2q8.astype(np.float32) * 0.5).astype(E4)   # exact halving
    wt = np.ascontiguousarray(w2h.transpose(1, 2, 0))  # [C1, K, C2]
    w2dr = np.zeros((C1, 2, 8 * C1), dtype=E4)
    for j in range(4):
        for i in range(2):
            for h in range(2):
                w2dr[:, i, (j * 2 + h) * C1 : (j * 2 + h + 1) * C1] = (
                    wt[:, 2 * j + i, h * C1 : (h + 1) * C1])
    w28 = np.ascontiguousarray(wt[:, 8, :])  # [C1, C2]
    # DC correction: E[s1[ci]] = Phi((b1 - TH1/G)/sigma), sigma = ||W1[ci]||
    sig = np.sqrt((W1.astype(np.float64) ** 2).sum(axis=(1, 2)))
    z = (b1.astype(np.float64) - TH1 / GAIN) / sig
    p_ci = np.array([0.5 * (1.0 + math.erf(v / math.sqrt(2.0))) for v in z])
    dw = w2q8.astype(np.float64) / (A2S * SC2) - W2.astype(np.float64)
    b2c = b2.astype(np.float64) - np.einsum("oik,i->o", dw, p_ci)
    # +/-1 encoding: psum = conv_q(s1) - S/2, S = sum of (quantized,
    # unhalved) weights over (ci, k); fold S/2 into b2p
    S = w2q8.astype(np.float64).sum(axis=(1, 2))
    nth1 = -(TH1 / GAIN - b1).reshape(C1, 1).astype(np.float32)
    b2p_full = (A2S * SC2 * b2c + 0.5 * S).astype(np.float32)
    b2p = b2p_full.reshape(2, C1).T.copy()            # [128, 2] cols = halves
    wfcT = Wfc.T.reshape(2, C1, NCLS)                 # [2, 128, 4]
    wfc_t = wfcT.transpose(1, 0, 2).reshape(C1, 2 * NCLS).copy()
    # counts = (sign_sum + T*L)/2 folded into the FC epilogue:
    # out = (Wfc @ sign_sum)/(2*T*L) + (bfc + 0.5*rowsum(Wfc))
    bfc_c = (bfc + 0.5 * Wfc.sum(axis=1)).reshape(NCLS, 1).astype(np.float32)
    return {
        "w1t": w1t.astype(ml_dtypes.bfloat16),
        "w2dr": w2dr,
        "w28": w28,
        "nth1": nth1,
        "b2p": b2p,
        "wfc": wfc_t.astype(np.float32),
        "bfc": bfc_c,
    }


def kernel(x, W1, b1, W2, b2, Wfc, bfc, _trace=False):
    x = np.asarray(x, dtype=np.float32)
    # [B, Cin, L, T] -> [B, Cin, T, L] bf16 so on-chip reads are unit-stride
    x_t = np.ascontiguousarray(x.transpose(0, 1, 3, 2)).astype(ml_dtypes.bfloat16)
    consts = _prep_consts(
        np.asarray(W1, np.float32), np.asarray(b1, np.float32),
        np.asarray(W2, np.float32), np.asarray(b2, np.float32),
        np.asarray(Wfc, np.float32), np.asarray(bfc, np.float32),
    )
    if "nc" not in _CACHE:
        _CACHE["nc"] = _build()
    nc = _CACHE["nc"]

    in_maps = []
    for c in range(N_CORES):
        m = dict(consts)
        m["x"] = np.ascontiguousarray(x_t[c * B_SH : (c + 1) * B_SH])
        in_maps.append(m)

    res = run_bass_kernel_spmd(
        nc, in_maps, core_ids=list(range(N_CORES)), trace=_trace
    )
    out = np.concatenate([res.results[c]["out"] for c in range(N_CORES)], axis=0)
    out = out.astype(np.float32)
    if _trace:
        return out, res
    return out


# revision 5
# speedup vs baseline: 4.1141x; 4.1141x over previous
"""Trainium2 Bass kernel for the ExportableStudentSNN1d problem (fp8 conv2).

Data-parallel over batch: 64 samples -> 8 cores x 8 samples. Each core runs
an identical NEFF on its batch shard; host concatenates the [8, 4] outputs.

Math notes (TAU1 = 1.0 makes layer-1 LIF memoryless):
  s1_t = (conv1(x_t)*G + b1*G >= TH1)        <=> conv1(x_t) >= TH1/G - b1
  s1 is stored +/-1-encoded: sgn = Sign(conv1 - th1) (ACT engine), and
  since s1 = (sgn+1)/2 is linear, conv2 runs on sgn with HALVED weights
  (exact power-of-2 scale in fp8) and 0.5*sum(W2q) folded into b2p.
  layer2 runs in a 2^11-scaled domain (all linear + thresholds scale):
     psum2 = conv2 with W2 pre-scaled by (10/9)*G*2^11/2, fp8-e4m3
     y     = (psum2 + b2p) + carry           (b2p, carry, TH2 all 2^11-scaled)
     m'    = (y < TH2') * (-1/9);  carry' = y * m'
     spike counts via ACT Sign(y - TH2') accumulation.
  fp8 weight-quantization DC error is cancelled by folding
  -sum_{ci,k} dW[co,ci,k] * E[s1[ci]] into b2 (E[s1[ci]] analytic: conv1
  output is Gaussian with sigma = ||W1[ci]||_F).
  out[b,c] = (sum_{t,l} sp)/(T*L) @ Wfc.T + bfc

conv2 uses fp8 DoubleRow matmuls: taps (0,1),(2,3),(4,5),(6,7) run as 4
pair-matmuls (2 MACs/cell/cycle), tap 8 as a normal fp8 matmul. The s1
tile holds two slabs [128, 2, S1P]: slab1 = slab0 shifted one column (DMA
SBUF->SBUF copy), so a pair's two taps read the same offset in both slabs.
conv1 stays bf16 with a DMA-materialized im2col.

Engine budget per (seg,t) (keeps every engine under the ~5.2us Tensor
time): DVE does y-stt + carry-mult per 512-chunk, GpSimd the m mask op per
chunk, ACT the two spike-count Signs + the s1 slab0 Sign, DMA the slab1
shift copy.
"""

import math

import numpy as np
import ml_dtypes

import concourse.bacc as bacc
import concourse.tile as tile
import concourse.mybir as mybir
from concourse.bass_utils import run_bass_kernel_spmd

F32 = mybir.dt.float32
BF16 = mybir.dt.bfloat16
FP8 = mybir.dt.float8e4
E4 = ml_dtypes.float8_e4m3

N_CORES = 8
B, C_IN, L, T = 64, 12, 2048, 20
C1, C2, K, PAD = 128, 256, 9, 4
GAIN, TAU2, TH1, TH2 = 3.0, 0.9, 0.02, 0.02
NCLS = 4
B_SH = B // N_CORES            # 8 samples per core
LH = 1024                      # L processed in halves
HALO = 8                       # x halo per side (conv1 then conv2 shifts)
S1W = LH + 2 * PAD             # 1032 s1 columns needed per L-half
S1P = 1040                     # s1 slab width (mult of 16 for DR pair step)
XW = LH + 2 * HALO             # 1040 x columns staged per L-half
A2S = (10.0 / 9.0) * GAIN      # 10/3: multiplier on conv2 psum
SC2 = 2.0 ** 11                # fp8 range scale; layer-2 runs 2^11-scaled
TH2S = TH2 * SC2
MDECAY = 1.0 / 9.0

_CACHE = {}


def _build():
    nc = bacc.Bacc("TRN2", target_bir_lowering=False, debug=False)

    x_d = nc.dram_tensor("x", [B_SH, C_IN, T, L], BF16, kind="ExternalInput")
    w1t_d = nc.dram_tensor("w1t", [K * C_IN, C1], BF16, kind="ExternalInput")
    w2dr_d = nc.dram_tensor("w2dr", [C1, 2, 8 * C1], FP8, kind="ExternalInput")
    w28_d = nc.dram_tensor("w28", [C1, C2], FP8, kind="ExternalInput")
    nth1_d = nc.dram_tensor("nth1", [C1, 1], F32, kind="ExternalInput")
    b2p_d = nc.dram_tensor("b2p", [C1, 2], F32, kind="ExternalInput")
    wfc_d = nc.dram_tensor("wfc", [C1, 2 * NCLS], F32, kind="ExternalInput")
    bfc_d = nc.dram_tensor("bfc", [NCLS, 1], F32, kind="ExternalInput")
    out_d = nc.dram_tensor("out", [B_SH, NCLS], F32, kind="ExternalOutput")

    with tile.TileContext(nc) as tc:
        with (
            tc.tile_pool(name="const", bufs=1) as cpool,
            tc.tile_pool(name="xstage", bufs=2) as xpool,
            tc.tile_pool(name="s1", bufs=2) as s1pool,
            tc.tile_pool(name="lif", bufs=3) as lifpool,
            tc.tile_pool(name="carry", bufs=2) as cpool2,
            tc.tile_pool(name="psum1", bufs=1, space="PSUM") as pp1,
            tc.tile_pool(name="psum2", bufs=2, space="PSUM") as pp2,
            tc.tile_pool(name="psfc", bufs=1, space="PSUM") as ppfc,
        ):
            # ---- constants / weights (resident) ----
            # w1t rows (12k+ci) hold W1[:, ci, k] (im2col layout)
            w1t = cpool.tile([K * C_IN, C1], BF16)
            nc.sync.dma_start(w1t[:], w1t_d.ap())
            # DR pairs: w2dr[ci, i, (j*2+h)*128+co] = W2s[h*128+co, ci, 2j+i]
            w2dr = cpool.tile([C1, 2, 8 * C1], FP8)
            nc.sync.dma_start(w2dr[:], w2dr_d.ap())
            # tap 8: w28[ci, h*128+co] = W2s[h*128+co, ci, 8]
            w28 = cpool.tile([C1, C2], FP8)
            nc.sync.dma_start(w28[:], w28_d.ap())
            nth1 = cpool.tile([C1, 1], F32)
            nc.sync.dma_start(nth1[:], nth1_d.ap())
            b2p = cpool.tile([C1, 2], F32)
            nc.sync.dma_start(b2p[:], b2p_d.ap())
            nth2 = cpool.tile([C1, 1], F32)
            nc.gpsimd.memset(nth2[:], -TH2S)
            wfc = cpool.tile([C1, 2 * NCLS], F32)
            nc.sync.dma_start(wfc[:], wfc_d.ap())
            bfc = cpool.tile([NCLS, 1], F32)
            nc.sync.dma_start(bfc[:], bfc_d.ap())
            # spike counts, one column per (h, b, lh, t)
            acc = cpool.tile([C1, 2 * B_SH * 2 * T], F32)

            segs = [(b, lh) for b in range(B_SH) for lh in range(2)]

            def stage_segment(idx):
                # im2col staging: rows (12k+ci) = x[ci] shifted by tap k.
                # column (t, c) of row-group k = x[b, ci, t, l0+c+k-8]
                b, lh = segs[idx]
                l0 = lh * LH
                xs = xpool.tile([K * C_IN, T * S1W], BF16)
                xsv = xs[:].rearrange("p (t c) -> p t c", c=S1W)
                # zero the possible halo bands (32-aligned base partition
                # required for engine ops -> memset all rows; the DMAs
                # below overwrite whatever is valid)
                if l0 == 0:
                    nc.gpsimd.memset(xsv[:, :, 0:HALO], 0.0)
                if l0 + LH == L:
                    nc.gpsimd.memset(xsv[:, :, S1W - HALO : S1W], 0.0)
                for k in range(K):
                    rows = slice(C_IN * k, C_IN * (k + 1))
                    c_lo = max(0, HALO - k - l0)
                    c_hi = min(S1W, L - l0 - k + HALO)
                    src = x_d.ap()[b, :, :,
                                   l0 + c_lo + k - HALO : l0 + c_hi + k - HALO]
                    if idx == 0:
                        # cold start: split so the first timesteps' columns
                        # land first
                        nc.sync.dma_start(
                            xsv[rows, 0:2, c_lo:c_hi], src[:, 0:2, :])
                        nc.sync.dma_start(
                            xsv[rows, 2:T, c_lo:c_hi], src[:, 2:T, :])
                    else:
                        nc.sync.dma_start(xsv[rows, :, c_lo:c_hi], src)
                carry = cpool2.tile([C1, 2 * LH], F32)
                nc.gpsimd.memset(carry[:], 0.0)
                return xs, carry

            def conv1_block(xs, t):
                # conv1: one K=108 matmul per chunk. s1 slab0 = Sign(p1-th1)
                # on ACT (+/-1 encoding); slab1 = slab0 shifted one column
                # via SBUF->SBUF DMA. Cols >= 1032 are junk (stale psum ->
                # +/-1) but are never read by the conv2 matmuls.
                p1 = pp1.tile([C1, 1536], F32)
                for c0, cn in ((0, 512), (512, 512), (1024, S1W - 1024)):
                    nc.tensor.matmul(
                        p1[:, c0 : c0 + cn],
                        w1t[:],
                        xs[:, t * S1W + c0 : t * S1W + c0 + cn],
                        start=True,
                        stop=True,
                    )
                s1 = s1pool.tile([C1, 2, S1P], FP8)
                nc.scalar.activation(
                    s1[:, 0], p1[:, 0:S1P],
                    mybir.ActivationFunctionType.Sign,
                    bias=nth1[:],
                )
                nc.sync.dma_start(s1[:, 1, 0 : S1P - 1], s1[:, 0, 1:S1P])
                return s1

            def conv2_block(s1, h):
                p2 = pp2.tile([C1, LH], F32)
                for c0 in (0, 512):
                    for j in range(4):
                        nc.tensor.matmul(
                            p2[:, c0 : c0 + 512],
                            w2dr[:, 0:2, (j * 2 + h) * C1 : (j * 2 + h + 1) * C1],
                            s1[:, 0:2, c0 + 2 * j : c0 + 2 * j + 512],
                            start=(j == 0),
                            stop=False,
                            perf_mode=mybir.MatmulPerfMode.DoubleRow,
                        )
                    nc.tensor.matmul(
                        p2[:, c0 : c0 + 512],
                        w28[:, h * C1 : (h + 1) * C1],
                        s1[:, 0, c0 + 8 : c0 + 8 + 512],
                        start=False,
                        stop=True,
                    )
                return p2

            def lif_front(carry, p2, h):
                # per 512-chunk: y = (psum2 + b2p) + carry (DVE),
                # m = (y < TH2')*(-1/9) (DVE; gpsimd's 2-op ts is ~8us!).
                # Chunked so the y->m->carry chain pipelines under the PE.
                ch = carry[:, h * LH : (h + 1) * LH]
                y = lifpool.tile([C1, LH], F32, tag="y")
                m = lifpool.tile([C1, LH], F32, tag="m")
                for q in (0, 512):
                    nc.vector.scalar_tensor_tensor(
                        y[:, q : q + 512], p2[:, q : q + 512],
                        b2p[:, h : h + 1], ch[:, q : q + 512],
                        op0=mybir.AluOpType.add, op1=mybir.AluOpType.add,
                    )
                    nc.vector.tensor_scalar(
                        m[:, q : q + 512], y[:, q : q + 512], TH2S, -MDECAY,
                        op0=mybir.AluOpType.is_lt, op1=mybir.AluOpType.mult,
                    )
                return y, m

            def lif_back(carry, y, m, h, col):
                # sign-sum for spike counting (off the carry chain)
                sg = lifpool.tile([C1, LH], F32, tag="sg")
                nc.scalar.activation(
                    sg[:], y[:], mybir.ActivationFunctionType.Sign,
                    bias=nth2[:],
                    accum_out=acc[:, col + h * (B_SH * 2 * T) :
                                  col + h * (B_SH * 2 * T) + 1],
                )
                # carry chain tail: h0 on gpsimd (its plain multiply is its
                # one fast op), h1 on DVE -- balances the two engines
                ch = carry[:, h * LH : (h + 1) * LH]
                eng = nc.gpsimd if h == 0 else nc.vector
                for q in (0, 512):
                    eng.tensor_tensor(
                        ch[:, q : q + 512], y[:, q : q + 512],
                        m[:, q : q + 512], op=mybir.AluOpType.mult,
                    )

            # conv1 of segment idx+1's t=0 fills the empty t=19 pipeline
            # slot of segment idx, so segment boundaries don't stall PE
            staged = stage_segment(0)
            s1_cur = conv1_block(staged[0], 0)
            for idx in range(len(segs)):
                b, lh = segs[idx]
                xs, carry = staged
                if idx + 1 < len(segs):
                    staged = stage_segment(idx + 1)
                for t in range(T):
                    col = b * (2 * T) + lh * T + t
                    p2_0 = conv2_block(s1_cur, 0)
                    # emit conv1(t+1)+s1(t+1) before the h0 LIF ops: PE order
                    # is unchanged, but s1 lands ~2us earlier so conv2(t+1,h0)
                    # never waits on it
                    if t + 1 < T:
                        s1_next = conv1_block(xs, t + 1)
                    elif idx + 1 < len(segs):
                        s1_next = conv1_block(staged[0], 0)
                    else:
                        s1_next = None
                    y0, m0 = lif_front(carry, p2_0, 0)
                    lif_back(carry, y0, m0, 0, col)
                    p2_1 = conv2_block(s1_cur, 1)
                    y1, m1 = lif_front(carry, p2_1, 1)
                    lif_back(carry, y1, m1, 1, col)
                    s1_cur = s1_next

            # ---- pooling + FC head ----
            pooled = cpool.tile([C1, 2 * B_SH], F32)
            nc.vector.tensor_reduce(
                pooled[:],
                acc[:].rearrange("p (h b c) -> p (h b) c", h=2, b=B_SH),
                axis=mybir.AxisListType.X, op=mybir.AluOpType.add,
            )
            pfc = ppfc.tile([NCLS, B_SH], F32)
            for h in range(2):
                nc.tensor.matmul(
                    pfc[:],
                    wfc[:, h * NCLS : (h + 1) * NCLS],
                    pooled[:, h * B_SH : (h + 1) * B_SH],
                    start=(h == 0),
                    stop=(h == 1),
                )
            # pfc holds Wfc @ sign_sums; counts = (sign_sum + T*L)/2 is folded
            # into scale and the host-adjusted bias
            fin = cpool.tile([NCLS, B_SH], F32)
            nc.scalar.activation(
                fin[:], pfc[:], mybir.ActivationFunctionType.Identity,
                bias=bfc[:], scale=1.0 / float(2 * T * L),
            )
            nc.sync.dma_start(out_d.ap().rearrange("b c -> c b"), fin[:])

    nc.compile()
    return nc


def _prep_consts(W1, b1, W2, b2, Wfc, bfc):
    # w1t im2col layout: row (12k+ci), col co = W1[co, ci, k]
    w1t = np.ascontiguousarray(W1.transpose(2, 1, 0)).reshape(K * C_IN, C1)
    # W2 pre-scaled to the 2^11 domain, fp8-e4m3 quantized, then HALVED
    # (exact in fp8) for the +/-1 s1 encoding
    w2q8 = (W2.astype(np.float64) * (A2S * SC2)).astype(np.float32).astype(E4)
    w2h = (w2q8.astype(np.float32) * 0.5).astype(E4)   # exact halving
    wt = np.ascontiguousarray(w2h.transpose(1, 2, 0))  # [C1, K, C2]
    w2dr = np.zeros((C1, 2, 8 * C1), dtype=E4)
    for j in range(4):
        for i in range(2):
            for h in range(2):
                w2dr[:, i, (j * 2 + h) * C1 : (j * 2 + h + 1) * C1] = (
                    wt[:, 2 * j + i, h * C1 : (h + 1) * C1])
    w28 = np.ascontiguousarray(wt[:, 8, :])  # [C1, C2]
    # DC correction: E[s1[ci]] = Phi((b1 - TH1/G)/sigma), sigma = ||W1[ci]||
    sig = np.sqrt((W1.astype(np.float64) ** 2).sum(axis=(1, 2)))
    z = (b1.astype(np.float64) - TH1 / GAIN) / sig
    p_ci = np.array([0.5 * (1.0 + math.erf(v / math.sqrt(2.0))) for v in z])
    dw = w2q8.astype(np.float64) / (A2S * SC2) - W2.astype(np.float64)
    b2c = b2.astype(np.float64) - np.einsum("oik,i->o", dw, p_ci)
    # +/-1 encoding: psum = conv_q(s1) - S/2, S = sum of (quantized,
    # unhalved) weights over (ci, k); fold S/2 into b2p
    S = w2q8.astype(np.float64).sum(axis=(1, 2))
    nth1 = -(TH1 / GAIN - b1).reshape(C1, 1).astype(np.float32)
    b2p_full = (A2S * SC2 * b2c + 0.5 * S).astype(np.float32)
    b2p = b2p_full.reshape(2, C1).T.copy()            # [128, 2] cols = halves
    wfcT = Wfc.T.reshape(2, C1, NCLS)                 # [2, 128, 4]
    wfc_t = wfcT.transpose(1, 0, 2).reshape(C1, 2 * NCLS).copy()
    # counts = (sign_sum + T*L)/2 folded into the FC epilogue:
    # out = (Wfc @ sign_sum)/(2*T*L) + (bfc + 0.5*rowsum(Wfc))
    bfc_c = (bfc + 0.5 * Wfc.sum(axis=1)).reshape(NCLS, 1).astype(np.float32)
    return {
        "w1t": w1t.astype(ml_dtypes.bfloat16),
        "w2dr": w2dr,
        "w28": w28,
        "nth1": nth1,
        "b2p": b2p,
        "wfc": wfc_t.astype(np.float32),
        "bfc": bfc_c,
    }


def kernel(x, W1, b1, W2, b2, Wfc, bfc, _trace=False):
    x = np.asarray(x, dtype=np.float32)
    # [B, Cin, L, T] -> [B, Cin, T, L] bf16 so on-chip reads are unit-stride
    x_t = np.ascontiguousarray(x.transpose(0, 1, 3, 2)).astype(ml_dtypes.bfloat16)
    consts = _prep_consts(
        np.asarray(W1, np.float32), np.asarray(b1, np.float32),
        np.asarray(W2, np.float32), np.asarray(b2, np.float32),
        np.asarray(Wfc, np.float32), np.asarray(bfc, np.float32),
    )
    if "nc" not in _CACHE:
        _CACHE["nc"] = _build()
    nc = _CACHE["nc"]

    in_maps = []
    for c in range(N_CORES):
        m = dict(consts)
        m["x"] = np.ascontiguousarray(x_t[c * B_SH : (c + 1) * B_SH])
        in_maps.append(m)

    res = run_bass_kernel_spmd(
        nc, in_maps, core_ids=list(range(N_CORES)), trace=_trace
    )
    out = np.concatenate([res.results[c]["out"] for c in range(N_CORES)], axis=0)
    out = out.astype(np.float32)
    if _trace:
        return out, res
    return out
